# revision 15
# baseline (speedup 1.0000x reference)
"""NT-Xent contrastive loss (forward) on 8 TRN2 NeuronCores via Bass/Tile.

Math: with h = concat(h_i, h_j) [N=8192, D=256], sim = (h @ h.T) / 0.5,
loss = mean_r( logsumexp_j(sim[r, j], j != r) - pos_r ), where
pos_r = sim[r, partner(r)].  The loss separates:
loss = (sum_r lse_r - 4 * sum(h_i * h_j)) / N; the pos term is a single
1M-element dot the host computes exactly in float64.

Sharding: core c owns rows [1024c, 1024c + 1024).  Each core receives the
full transposed h, column-rotated by its row offset, so one SPMD program
serves all 8 cores: the self-similarity diagonal lands at core-invariant
positions.

Per core: the PE builds each 128-row block of sim in PSUM from fp8 e4m3
operands with the DoubleRow perf mode (2 fp8 MACs/cell/cycle, fp32
accumulate); the whole 8-bank PSUM is one hand-managed ring tile, 16
bank-chunks per row-block in two 8-bank halves.  Per half: banks 0-2 go
to the DVE as a u16/bfloat16 Schraudolph bit-trick exp — a 2-bank + 1-bank
tensor_scalar pair (u16 bits; the split shortens the drain->refill->drain
critical loop) followed by one fused scalar_tensor_tensor that folds the
two bf16 halves and row-sums them via accum_out.  Banks 3-7 go to the
scalar engine as exp(2x - M_row) with a fused row-sum, split into a
3-bank + 2-bank activation pair so the next half's refills overlap the
second one (a single 5-bank activation serializes the ring via its
write-after-read chain).  The diagonal is masked by accumulating
I.T @ (-1e9 shifted-diag) as an extra bf16 matmul.  Each core emits a
[128, 48] tile of partial sums; the host finishes with log/sum in
float64.  M is a runtime input (per-row); if a row's exp-sum
under/overflows fp32, the host retries with a shifted M for those rows.
"""

import numpy as np
import ml_dtypes

B = 4096
D = 256
N = 2 * B            # 8192 rows/cols of sim
NCORES = 8
KCH = D // 128       # 2 contraction chunks of 128
NRB = 8              # row-blocks of 128 per core
M_DEFAULT = 161.0
MASK_NEG = -1.0e9
WINC = 4608          # computed cols per row-block (4224 real + 384 pad)
HTW = 7 * 1024 + WINC  # 11776: extended (wrapped) rotated h.T width
NSLOT = 9            # colsum PSUM slots (3 banks x 3 positions)
NM = 22              # distinct extended 512-col colsum chunks

# u16/bf16 Schraudolph: exp(y) ~= bitcast_bf16(u16(round(A16*y + B16)))
EXP_A16 = float(2 ** 7 / np.log(2.0))
EXP_B16 = 16248.5537

# chunk j (0..8) -> physical 512-col slot of the 5-bank sim ring
PHYS = [0, 1, 2, 3, 4, 0, 1, 2, 3]

TRACE = False
LAST_RESULTS = None

_cache = {}


def _mslots():
    """Per colsum chunk m: (first_rb, last_rb). rb k touches m = 2k..2k+7."""
    first = {m: max(0, -(-(m - 7) // 2)) for m in range(NM)}
    last = {m: min(7, m // 2) for m in range(NM)}
    return first, last


def _build():
    if "nc" in _cache:
        return _cache["nc"]

    import concourse.tile as tile
    import concourse.mybir as mybir
    from concourse import bacc

    f32 = mybir.dt.float32
    bf16 = mybir.dt.bfloat16
    fp8 = mybir.dt.float8e4
    u16 = mybir.dt.uint16
    DR = mybir.MatmulPerfMode.DoubleRow
    Exp = mybir.ActivationFunctionType.Exp

    kf, kl = _mslots()

    nc = bacc.Bacc("TRN2", target_bir_lowering=False, num_devices=NCORES)
    ht_dram = nc.dram_tensor("ht", [KCH, 128, HTW], fp8, kind="ExternalInput").ap()
    eye_dram = nc.dram_tensor("eye", [1, 128, 128], bf16, kind="ExternalInput").ap()
    maskd_dram = nc.dram_tensor("maskd", [128, 512], bf16, kind="ExternalInput").ap()
    maskt_dram = nc.dram_tensor("maskt", [128, 512], bf16, kind="ExternalInput").ap()
    ones_dram = nc.dram_tensor("ones32", [128, 32], bf16, kind="ExternalInput").ap()
    out_dram = nc.dram_tensor("out", [128, 32], f32, kind="ExternalOutput").ap()
    outc_dram = nc.dram_tensor("outc", [8, 128, 512], f32, kind="ExternalOutput").ap()

    with tile.TileContext(nc) as tc:
        with (
            tc.tile_pool(name="hpool", bufs=1) as hpool,
            tc.tile_pool(name="small", bufs=1) as small,
            tc.tile_pool(name="epool", bufs=2) as epool,
            tc.tile_pool(name="spool", bufs=2) as spool,
            tc.tile_pool(name="psum", bufs=1, space="PSUM") as psum,
        ):
            eye_pos = small.tile([128, 128], bf16)
            nc.gpsimd.dma_start(out=eye_pos, in_=eye_dram[0])
            maskd_sb = small.tile([128, 512], bf16)
            nc.gpsimd.dma_start(out=maskd_sb, in_=maskd_dram)
            maskt_sb = small.tile([128, 512], bf16)
            nc.gpsimd.dma_start(out=maskt_sb, in_=maskt_dram)
            ones_sb = small.tile([128, 32], bf16)
            nc.gpsimd.dma_start(out=ones_sb, in_=ones_dram)

            bias_sb = small.tile([128, 1], f32)
            nc.vector.memset(bias_sb, -M_DEFAULT)
            warm_sb = small.tile([128, 1], f32)
            nc.scalar.activation(
                out=warm_sb, in_=ones_sb[:, 0:1], func=Exp,
                bias=bias_sb[:, 0:1], scale=0.0,
            )

            ring = psum.tile([128, 2560], f32, name="ring")   # 5 banks
            cs = psum.tile([128, 1536], f32, name="cs")       # 3 banks

            # PE clock warm-up during the DMA prologue.
            # (the bias memset above also triggers the DVE uop-table load
            # during the DMA prologue)
            wsrc = small.tile([128, 128], bf16)
            nc.gpsimd.memset(wsrc, 0.0)
            for w in range(12):
                nc.tensor.matmul(
                    ring[:, 0:128], lhsT=wsrc, rhs=wsrc, start=True, stop=True,
                )

            col_ranges = [(0, 1024), (1024, 2560), (2560, 4608), (4608, 6656),
                          (6656, 8704), (8704, 10752), (10752, 11776)]
            ht_tiles = []
            for c0, c1 in col_ranges:
                t = hpool.tile([128, KCH, c1 - c0], fp8, name=f"ht_{c0}")
                nc.sync.dma_start(
                    out=t,
                    in_=ht_dram[:, :, c0:c1].rearrange("k p c -> p k c"),
                )
                ht_tiles.append(t)

            def ht_slice(c0, w):
                for (r0, r1), t in zip(col_ranges, ht_tiles):
                    if r0 <= c0 < r1:
                        assert c0 + w <= r1, (c0, w)
                        return t[:, :, c0 - r0:c0 - r0 + w]
                raise AssertionError(c0)

            res_sb = small.tile([128, 32], f32)
            e_tiles = [None] * NRB

            def cs_out(m):
                s = m % NSLOT
                p0 = 32 * (s % 3)
                c0 = 512 * (s // 3)
                return cs[p0:p0 + 32, c0:c0 + 512]

            def emit_evac(e, bank):
                st = spool.tile([128, 512], f32, name="stage")
                nc.scalar.copy(st, cs[:, 512 * bank:512 * bank + 512])
                nc.sync.dma_start(out=outc_dram[e], in_=st)

            # colsum slot-bank evacuations, keyed by the m whose start=True
            # reuses the bank: evac must precede that matmul.
            EVAC_BEFORE = {9: (0, 0), 12: (1, 1), 15: (2, 2)}
            EVAC_BEFORE2 = {18: (3, 0), 21: (4, 1)}

            def emit_colsums(k):
                E = e_tiles[k]
                for j in range(8):
                    m = 2 * k + j
                    if k == kf[m]:
                        ev = EVAC_BEFORE.get(m) or EVAC_BEFORE2.get(m)
                        if ev is not None:
                            emit_evac(*ev)
                    nc.tensor.matmul(
                        cs_out(m),
                        lhsT=ones_sb,
                        rhs=E[:, 512 * j:512 * j + 512],
                        start=(k == kf[m]),
                        stop=(k == kl[m]),
                    )

            for k in range(NRB):
                base = 1024 * k
                lhsT = ht_slice(base, 128)
                E = epool.tile([128, WINC], bf16, name="E")
                e_tiles[k] = E

                def sim_chunk(j, start=True, stop=True):
                    nc.tensor.matmul(
                        ring[:, 512 * PHYS[j]:512 * PHYS[j] + 512],
                        lhsT=lhsT,
                        rhs=ht_slice(base + 512 * j, 512),
                        start=start,
                        stop=stop,
                        perf_mode=DR,
                    )

                # D1 (chunks j0-j1, diag-masked) -> DVE u16 Schraudolph
                nc.tensor.matmul(
                    ring[:, 0:512], lhsT=eye_pos, rhs=maskd_sb,
                    start=True, stop=False,
                )
                sim_chunk(0, start=False)
                sim_chunk(1)
                ti_a = E[:, 0:1024].bitcast(u16)
                nc.vector.tensor_scalar(
                    ti_a, ring[:, 0:1024], 2.0 * EXP_A16,
                    EXP_B16 - EXP_A16 * M_DEFAULT,
                    mybir.AluOpType.mult, mybir.AluOpType.add,
                )
                nc.vector.reduce_sum(
                    res_sb[:, 4 * k + 2:4 * k + 3], E[:, 0:1024],
                    axis=mybir.AxisListType.X,
                )
                # diag-block exps must not enter the column sums
                nc.gpsimd.memset(E[:, 0:128], 0.0)
                # A1 (chunks j2-j4) -> ACT exp with fused row-sum
                sim_chunk(2)
                sim_chunk(3)
                sim_chunk(4)
                nc.scalar.activation(
                    out=E[:, 1024:2560], in_=ring[:, 1024:2560], func=Exp,
                    bias=bias_sb[:, 0:1], scale=2.0,
                    accum_out=res_sb[:, 4 * k:4 * k + 1],
                )
                # A2 (chunks j5-j7 reuse phys banks 0-2) -> ACT
                sim_chunk(5)
                sim_chunk(6)
                sim_chunk(7)
                nc.scalar.activation(
                    out=E[:, 2560:4096], in_=ring[:, 0:1536], func=Exp,
                    bias=bias_sb[:, 0:1], scale=2.0,
                    accum_out=res_sb[:, 4 * k + 1:4 * k + 2],
                )
                # T (pad-masked chunk j8 reuses phys bank 3) -> DVE
                nc.tensor.matmul(
                    ring[:, 1536:2048], lhsT=eye_pos, rhs=maskt_sb,
                    start=True, stop=False,
                )
                sim_chunk(8, start=False)
                ti_b = E[:, 4096:4608].bitcast(u16)
                nc.vector.tensor_scalar(
                    ti_b, ring[:, 1536:2048], 2.0 * EXP_A16,
                    EXP_B16 - EXP_A16 * M_DEFAULT,
                    mybir.AluOpType.mult, mybir.AluOpType.add,
                )
                nc.vector.reduce_sum(
                    res_sb[:, 4 * k + 3:4 * k + 4], E[:, 4096:4608],
                    axis=mybir.AxisListType.X,
                )
                # column sums of the previous row-block's exp tile
                if k > 0:
                    emit_colsums(k - 1)

            emit_colsums(7)
            emit_evac(5, 2)
            emit_evac(6, 0)
            emit_evac(7, 1)
            nc.sync.dma_start(out=out_dram, in_=res_sb)

    nc.compile()
    _cache["nc"] = nc
    return nc


def _make_static_inputs(h_i, h_j):
    h = np.concatenate([np.asarray(h_i), np.asarray(h_j)], axis=0).astype(np.float32)
    hT = np.ascontiguousarray(h.T)  # [256, 8192]
    np.clip(hT, -240.0, 240.0, out=hT)
    hq8 = hT.astype(ml_dtypes.float8_e4m3)
    hts = []
    for c in range(NCORES):
        rot = np.roll(hq8, -128 * c, axis=1)
        ext = np.concatenate([rot, rot[:, :HTW - N]], axis=1)
        hts.append(np.ascontiguousarray(ext.reshape(KCH, 128, HTW)))
    p = np.arange(128)
    eye = np.zeros((1, 128, 128), dtype=ml_dtypes.bfloat16)
    eye[0, p, p] = 1.0
    maskd = np.zeros((128, 512), dtype=np.float32)
    maskd[p, p] = MASK_NEG
    maskt = np.zeros((128, 512), dtype=np.float32)
    maskt[:, 128:] = MASK_NEG
    ones32 = np.ones((128, 32), dtype=ml_dtypes.bfloat16)
    return (hts, eye, maskd.astype(ml_dtypes.bfloat16),
            maskt.astype(ml_dtypes.bfloat16), ones32,
            hq8.astype(np.float64).T)  # quantized h [8192, 256] f64


def _axon_reset():
    try:
        import ctypes

        lib = ctypes.CDLL("/opt/axon/libaxon_pjrt.so")
        lib.axon_reset.restype = ctypes.c_int64
        return lib.axon_reset() == 0
    except Exception:
        return False


def _run(nc, hts, eye, maskd, maskt, ones32):
    global LAST_RESULTS
    from concourse import bass_utils

    in_maps = [
        {"ht": hts[c], "eye": eye, "maskd": maskd, "maskt": maskt,
         "ones32": ones32}
        for c in range(NCORES)
    ]
    try:
        results = bass_utils.run_bass_kernel_spmd(
            nc, in_maps, core_ids=list(range(NCORES)), trace=TRACE
        )
    except Exception:
        if not _axon_reset():
            raise
        results = bass_utils.run_bass_kernel_spmd(
            nc, in_maps, core_ids=list(range(NCORES)), trace=TRACE
        )
    LAST_RESULTS = results
    return results.results


def kernel(h_i, h_j):
    nc = _build()
    hts, eye, maskd, maskt, ones32, hq = _make_static_inputs(h_i, h_j)
    res = _run(nc, hts, eye, maskd, maskt, ones32)

    kf, kl = _mslots()
    M = M_DEFAULT
    S = np.zeros(N)
    for c in range(NCORES):
        out = res[c]["out"].astype(np.float64)     # [128, 32]
        outc = res[c]["outc"].astype(np.float64)   # [6, 128, 512]
        # row sums: rb k owns global rows [128*(c+8k), +128)
        Srb = out.reshape(128, NRB, 4).sum(axis=2)
        for k in range(NRB):
            g = c + 8 * k
            S[128 * g:128 * g + 128] += Srb[:, k]
        # column sums: evac e holds one cs bank; m -> (evac, position)
        V = np.zeros((NM, 512))
        for m in range(NM):
            e, pos = m // 3, m % 3
            V[m] = outc[e, 32 * pos, :]
        Vx = V.reshape(-1)            # ext cols [0, 11264)
        Vx[:128] = 0.0
        Vg = Vx[:N].copy()
        Vg[:Vx.shape[0] - N] += Vx[N:]
        S += np.roll(Vg, 128 * c)

    lse = np.full(N, np.nan)
    ok = np.isfinite(S) & (S > 0.0)
    lse[ok] = M + np.log(S[ok])
    if not ok.all():
        bad = np.where(~ok)[0]
        simb = 2.0 * (hq[bad] @ hq.T)
        simb[np.arange(len(bad)), bad] = -np.inf
        mb = simb.max(1)
        lse[bad] = mb + np.log(np.exp(simb - mb[:, None]).sum(1))

    total_pd = float(
        np.sum(np.asarray(h_i, dtype=np.float64) * np.asarray(h_j, dtype=np.float64))
    )
    loss = (lse.sum() - 4.0 * total_pd) / float(N)
    return np.array(loss, dtype=np.float32)


if __name__ == "__main__":
    rng = np.random.default_rng(0)
    h_i = rng.standard_normal((B, D), dtype=np.float32)
    h_j = rng.standard_normal((B, D), dtype=np.float32)
    print("loss:", kernel(h_i, h_j))


# revision 16
# speedup vs baseline: 1.0192x; 1.0192x over previous
"""NT-Xent contrastive loss (forward) on 8 TRN2 NeuronCores via Bass/Tile.

Math: with h = concat(h_i, h_j) [N=8192, D=256], sim = (h @ h.T) / 0.5,
loss = mean_r( logsumexp_j(sim[r, j], j != r) - pos_r ), where
pos_r = sim[r, partner(r)].  The loss separates:
loss = (sum_r lse_r - 4 * sum(h_i * h_j)) / N; the pos term is a single
1M-element dot the host computes exactly in float64.

Sharding: core c owns rows [1024c, 1024c + 1024).  Each core receives the
full transposed h, column-rotated by its row offset, so one SPMD program
serves all 8 cores: the self-similarity diagonal lands at core-invariant
positions.

Per core: the PE builds each 128-row block of sim in PSUM from fp8 e4m3
operands with the DoubleRow perf mode (2 fp8 MACs/cell/cycle, fp32
accumulate); the whole 8-bank PSUM is one hand-managed ring tile, 16
bank-chunks per row-block in two 8-bank halves.  Per half: banks 0-2 go
to the DVE as a u16/bfloat16 Schraudolph bit-trick exp — a 2-bank + 1-bank
tensor_scalar pair (u16 bits; the split shortens the drain->refill->drain
critical loop) followed by one fused scalar_tensor_tensor that folds the
two bf16 halves and row-sums them via accum_out.  Banks 3-7 go to the
scalar engine as exp(2x - M_row) with a fused row-sum, split into a
3-bank + 2-bank activation pair so the next half's refills overlap the
second one (a single 5-bank activation serializes the ring via its
write-after-read chain).  The diagonal is masked by accumulating
I.T @ (-1e9 shifted-diag) as an extra bf16 matmul.  Each core emits a
[128, 48] tile of partial sums; the host finishes with log/sum in
float64.  M is a runtime input (per-row); if a row's exp-sum
under/overflows fp32, the host retries with a shifted M for those rows.
"""

import numpy as np
import ml_dtypes

B = 4096
D = 256
N = 2 * B            # 8192 rows/cols of sim
NCORES = 8
KCH = D // 128       # 2 contraction chunks of 128
NRB = 8              # row-blocks of 128 per core
M_DEFAULT = 161.0
MASK_NEG = -1.0e9
WINC = 4608          # computed cols per row-block (4224 real + 384 pad)
HTW = 7 * 1024 + WINC  # 11776: extended (wrapped) rotated h.T width
NSLOT = 9            # colsum PSUM slots (3 banks x 3 positions)
NM = 22              # distinct extended 512-col colsum chunks

# u16/bf16 Schraudolph: exp(y) ~= bitcast_bf16(u16(round(A16*y + B16)))
EXP_A16 = float(2 ** 7 / np.log(2.0))
EXP_B16 = 16248.5537

# chunk j (0..8) -> physical 512-col slot of the 5-bank sim ring
PHYS = [0, 1, 2, 3, 4, 0, 1, 2, 3]

TRACE = False
LAST_RESULTS = None

_cache = {}


def _mslots():
    """Per colsum chunk m: (first_rb, last_rb). rb k touches m = 2k..2k+7."""
    first = {m: max(0, -(-(m - 7) // 2)) for m in range(NM)}
    last = {m: min(7, m // 2) for m in range(NM)}
    return first, last


def _build():
    if "nc" in _cache:
        return _cache["nc"]

    import concourse.tile as tile
    import concourse.mybir as mybir
    from concourse import bacc

    f32 = mybir.dt.float32
    bf16 = mybir.dt.bfloat16
    fp8 = mybir.dt.float8e4
    u16 = mybir.dt.uint16
    DR = mybir.MatmulPerfMode.DoubleRow
    Exp = mybir.ActivationFunctionType.Exp

    kf, kl = _mslots()

    nc = bacc.Bacc("TRN2", target_bir_lowering=False, num_devices=NCORES)
    ht_dram = nc.dram_tensor("ht", [KCH, 128, HTW], fp8, kind="ExternalInput").ap()
    eye_dram = nc.dram_tensor("eye", [1, 128, 128], bf16, kind="ExternalInput").ap()
    maskd_dram = nc.dram_tensor("maskd", [128, 512], bf16, kind="ExternalInput").ap()
    maskt_dram = nc.dram_tensor("maskt", [128, 512], bf16, kind="ExternalInput").ap()
    ones_dram = nc.dram_tensor("ones32", [128, 32], bf16, kind="ExternalInput").ap()
    out_dram = nc.dram_tensor("out", [128, 32], f32, kind="ExternalOutput").ap()
    outc_dram = nc.dram_tensor("outc", [8, 128, 512], f32, kind="ExternalOutput").ap()

    with tile.TileContext(nc) as tc:
        with (
            tc.tile_pool(name="hpool", bufs=1) as hpool,
            tc.tile_pool(name="small", bufs=1) as small,
            tc.tile_pool(name="epool", bufs=2) as epool,
            tc.tile_pool(name="spool", bufs=2) as spool,
            tc.tile_pool(name="psum", bufs=1, space="PSUM") as psum,
        ):
            eye_pos = small.tile([128, 128], bf16)
            nc.gpsimd.dma_start(out=eye_pos, in_=eye_dram[0])
            maskd_sb = small.tile([128, 512], bf16)
            nc.gpsimd.dma_start(out=maskd_sb, in_=maskd_dram)
            maskt_sb = small.tile([128, 512], bf16)
            nc.gpsimd.dma_start(out=maskt_sb, in_=maskt_dram)
            ones_sb = small.tile([128, 32], bf16)
            nc.gpsimd.dma_start(out=ones_sb, in_=ones_dram)

            bias_sb = small.tile([128, 1], f32)
            nc.vector.memset(bias_sb, -M_DEFAULT)
            warm_sb = small.tile([128, 1], f32)
            nc.scalar.activation(
                out=warm_sb, in_=ones_sb[:, 0:1], func=Exp,
                bias=bias_sb[:, 0:1], scale=0.0,
            )

            ring = psum.tile([128, 2560], f32, name="ring")   # 5 banks
            cs = psum.tile([128, 1536], f32, name="cs")       # 3 banks

            # PE clock warm-up during the DMA prologue.
            wsrc = small.tile([128, 128], bf16)
            nc.gpsimd.memset(wsrc, 0.0)
            # touch the DVE early so its uop-table load hides in the
            # DMA prologue instead of delaying the first real drain
            nc.vector.memset(warm_sb, 0.0)
            for w in range(12):
                nc.tensor.matmul(
                    ring[:, 0:128], lhsT=wsrc, rhs=wsrc, start=True, stop=True,
                )

            col_ranges = [(0, 1024), (1024, 2560), (2560, 4608), (4608, 6656),
                          (6656, 8704), (8704, 10752), (10752, 11776)]
            ht_tiles = []
            for c0, c1 in col_ranges:
                t = hpool.tile([128, KCH, c1 - c0], fp8, name=f"ht_{c0}")
                nc.sync.dma_start(
                    out=t,
                    in_=ht_dram[:, :, c0:c1].rearrange("k p c -> p k c"),
                )
                ht_tiles.append(t)

            def ht_slice(c0, w):
                for (r0, r1), t in zip(col_ranges, ht_tiles):
                    if r0 <= c0 < r1:
                        assert c0 + w <= r1, (c0, w)
                        return t[:, :, c0 - r0:c0 - r0 + w]
                raise AssertionError(c0)

            res_sb = small.tile([128, 32], f32)
            e_tiles = [None] * NRB

            def cs_out(m):
                s = m % NSLOT
                p0 = 32 * (s % 3)
                c0 = 512 * (s // 3)
                return cs[p0:p0 + 32, c0:c0 + 512]

            def emit_evac(e, bank):
                st = spool.tile([128, 512], f32, name="stage")
                nc.scalar.copy(st, cs[:, 512 * bank:512 * bank + 512])
                nc.sync.dma_start(out=outc_dram[e], in_=st)

            # colsum slot-bank evacuations, keyed by the m whose start=True
            # reuses the bank: evac must precede that matmul.
            EVAC_BEFORE = {9: (0, 0), 12: (1, 1), 15: (2, 2)}
            EVAC_BEFORE2 = {18: (3, 0), 21: (4, 1)}

            def emit_colsums(k):
                E = e_tiles[k]
                for j in range(8):
                    m = 2 * k + j
                    if k == kf[m]:
                        ev = EVAC_BEFORE.get(m) or EVAC_BEFORE2.get(m)
                        if ev is not None:
                            emit_evac(*ev)
                    nc.tensor.matmul(
                        cs_out(m),
                        lhsT=ones_sb,
                        rhs=E[:, 512 * j:512 * j + 512],
                        start=(k == kf[m]),
                        stop=(k == kl[m]),
                    )

            for k in range(NRB):
                base = 1024 * k
                lhsT = ht_slice(base, 128)
                E = epool.tile([128, WINC], bf16, name="E")
                e_tiles[k] = E

                def sim_chunk(j, start=True, stop=True):
                    nc.tensor.matmul(
                        ring[:, 512 * PHYS[j]:512 * PHYS[j] + 512],
                        lhsT=lhsT,
                        rhs=ht_slice(base + 512 * j, 512),
                        start=start,
                        stop=stop,
                        perf_mode=DR,
                    )

                # D1 (chunks j0-j1, diag-masked) -> DVE u16 Schraudolph
                nc.tensor.matmul(
                    ring[:, 0:512], lhsT=eye_pos, rhs=maskd_sb,
                    start=True, stop=False,
                )
                sim_chunk(0, start=False)
                sim_chunk(1)
                ti_a = E[:, 0:1024].bitcast(u16)
                nc.vector.tensor_scalar(
                    ti_a, ring[:, 0:1024], 2.0 * EXP_A16,
                    EXP_B16 - EXP_A16 * M_DEFAULT,
                    mybir.AluOpType.mult, mybir.AluOpType.add,
                )
                nc.vector.reduce_sum(
                    res_sb[:, 4 * k + 2:4 * k + 3], E[:, 0:1024],
                    axis=mybir.AxisListType.X,
                )
                # diag-block exps must not enter the column sums
                nc.gpsimd.memset(E[:, 0:128], 0.0)
                # A1 (chunks j2-j4) -> ACT exp with fused row-sum
                sim_chunk(2)
                sim_chunk(3)
                sim_chunk(4)
                nc.scalar.activation(
                    out=E[:, 1024:2560], in_=ring[:, 1024:2560], func=Exp,
                    bias=bias_sb[:, 0:1], scale=2.0,
                    accum_out=res_sb[:, 4 * k:4 * k + 1],
                )
                # A2 (chunks j5-j7 reuse phys banks 0-2) -> ACT
                sim_chunk(5)
                sim_chunk(6)
                sim_chunk(7)
                nc.scalar.activation(
                    out=E[:, 2560:4096], in_=ring[:, 0:1536], func=Exp,
                    bias=bias_sb[:, 0:1], scale=2.0,
                    accum_out=res_sb[:, 4 * k + 1:4 * k + 2],
                )
                # T (pad-masked chunk j8 reuses phys bank 3) -> DVE
                nc.tensor.matmul(
                    ring[:, 1536:2048], lhsT=eye_pos, rhs=maskt_sb,
                    start=True, stop=False,
                )
                sim_chunk(8, start=False)
                ti_b = E[:, 4096:4608].bitcast(u16)
                nc.vector.tensor_scalar(
                    ti_b, ring[:, 1536:2048], 2.0 * EXP_A16,
                    EXP_B16 - EXP_A16 * M_DEFAULT,
                    mybir.AluOpType.mult, mybir.AluOpType.add,
                )
                nc.vector.reduce_sum(
                    res_sb[:, 4 * k + 3:4 * k + 4], E[:, 4096:4608],
                    axis=mybir.AxisListType.X,
                )
                # column sums of the previous row-block's exp tile
                if k > 0:
                    emit_colsums(k - 1)

            emit_colsums(7)
            emit_evac(5, 2)
            emit_evac(6, 0)
            emit_evac(7, 1)
            nc.sync.dma_start(out=out_dram, in_=res_sb)

    nc.compile()
    _cache["nc"] = nc
    return nc


def _make_static_inputs(h_i, h_j):
    h = np.concatenate([np.asarray(h_i), np.asarray(h_j)], axis=0).astype(np.float32)
    hT = np.ascontiguousarray(h.T)  # [256, 8192]
    np.clip(hT, -240.0, 240.0, out=hT)
    hq8 = hT.astype(ml_dtypes.float8_e4m3)
    hts = []
    for c in range(NCORES):
        rot = np.roll(hq8, -128 * c, axis=1)
        ext = np.concatenate([rot, rot[:, :HTW - N]], axis=1)
        hts.append(np.ascontiguousarray(ext.reshape(KCH, 128, HTW)))
    p = np.arange(128)
    eye = np.zeros((1, 128, 128), dtype=ml_dtypes.bfloat16)
    eye[0, p, p] = 1.0
    maskd = np.zeros((128, 512), dtype=np.float32)
    maskd[p, p] = MASK_NEG
    maskt = np.zeros((128, 512), dtype=np.float32)
    maskt[:, 128:] = MASK_NEG
    ones32 = np.ones((128, 32), dtype=ml_dtypes.bfloat16)
    return (hts, eye, maskd.astype(ml_dtypes.bfloat16),
            maskt.astype(ml_dtypes.bfloat16), ones32,
            hq8.astype(np.float64).T)  # quantized h [8192, 256] f64


def _axon_reset():
    try:
        import ctypes

        lib = ctypes.CDLL("/opt/axon/libaxon_pjrt.so")
        lib.axon_reset.restype = ctypes.c_int64
        return lib.axon_reset() == 0
    except Exception:
        return False


def _run(nc, hts, eye, maskd, maskt, ones32):
    global LAST_RESULTS
    from concourse import bass_utils

    in_maps = [
        {"ht": hts[c], "eye": eye, "maskd": maskd, "maskt": maskt,
         "ones32": ones32}
        for c in range(NCORES)
    ]
    try:
        results = bass_utils.run_bass_kernel_spmd(
            nc, in_maps, core_ids=list(range(NCORES)), trace=TRACE
        )
    except Exception:
        if not _axon_reset():
            raise
        results = bass_utils.run_bass_kernel_spmd(
            nc, in_maps, core_ids=list(range(NCORES)), trace=TRACE
        )
    LAST_RESULTS = results
    return results.results


def kernel(h_i, h_j):
    nc = _build()
    hts, eye, maskd, maskt, ones32, hq = _make_static_inputs(h_i, h_j)
    res = _run(nc, hts, eye, maskd, maskt, ones32)

    kf, kl = _mslots()
    M = M_DEFAULT
    S = np.zeros(N)
    for c in range(NCORES):
        out = res[c]["out"].astype(np.float64)     # [128, 32]
        outc = res[c]["outc"].astype(np.float64)   # [6, 128, 512]
        # row sums: rb k owns global rows [128*(c+8k), +128)
        Srb = out.reshape(128, NRB, 4).sum(axis=2)
        for k in range(NRB):
            g = c + 8 * k
            S[128 * g:128 * g + 128] += Srb[:, k]
        # column sums: evac e holds one cs bank; m -> (evac, position)
        V = np.zeros((NM, 512))
        for m in range(NM):
            e, pos = m // 3, m % 3
            V[m] = outc[e, 32 * pos, :]
        Vx = V.reshape(-1)            # ext cols [0, 11264)
        Vx[:128] = 0.0
        Vg = Vx[:N].copy()
        Vg[:Vx.shape[0] - N] += Vx[N:]
        S += np.roll(Vg, 128 * c)

    lse = np.full(N, np.nan)
    ok = np.isfinite(S) & (S > 0.0)
    lse[ok] = M + np.log(S[ok])
    if not ok.all():
        bad = np.where(~ok)[0]
        simb = 2.0 * (hq[bad] @ hq.T)
        simb[np.arange(len(bad)), bad] = -np.inf
        mb = simb.max(1)
        lse[bad] = mb + np.log(np.exp(simb - mb[:, None]).sum(1))

    total_pd = float(
        np.sum(np.asarray(h_i, dtype=np.float64) * np.asarray(h_j, dtype=np.float64))
    )
    loss = (lse.sum() - 4.0 * total_pd) / float(N)
    return np.array(loss, dtype=np.float32)


if __name__ == "__main__":
    rng = np.random.default_rng(0)
    h_i = rng.standard_normal((B, D), dtype=np.float32)
    h_j = rng.standard_normal((B, D), dtype=np.float32)
    print("loss:", kernel(h_i, h_j))
